# revision 4
# baseline (speedup 1.0000x reference)
"""Causal self-attention (B=2, T=2048, C=1024, H=16) on 8 trn2 NeuronCores.

Sharding: batch x head-group. Core c handles batch b = c//4 and heads
[4*(c%4), 4*(c%4)+4). Each core computes qkv for its head slice, causal
attention, and a partial c_proj ([T, C] over its 256 input rows of W_proj);
the host sums the 4 partials per batch (fp16 partials, fp32 host sum).

v2 schedule (vs v1's ~220us): the ACT engine's exp stream (~80us total,
1 elem/lane/cycle @1.2GHz, irreducible) is the co-critical resource next to
the PE's ~115us of matmul work.  v1 saturated ACT in its back half by also
putting psum->sbuf copies there; v2:
  - all output copies/adds ride DVE; the softmax-normalize chain is
    DVE-reciprocal (straight from psum) -> gpsimd partition_broadcast
    (attn ucode library) -> DVE multiply: no ACT, no DMA round-trips;
  - block order 2.0 qkv2 3.0 qkv3 0.0 1.0 | 2.1 3.1 0.1 1.1 spreads exp
    into the qkv phase; each m=1 block is braided with 512-col c_proj
    half-tiles so the PE always has dense filler while exp paces;
  - per-psum-bank split normalize: bank0 of an O accumulation is final at
    j=8m+3, so its half of yT normalizes (and the dependent proj tiles
    start) four j-steps before the block ends -- the 1.1 tail shrinks to
    a half-norm chain + 4 proj half-tiles;
  - input DMAs are issued round-robin over the sync/vector/scalar queues
    (queue issue is 565-667ns per dma_start -- one queue serializes to
    ~18us); the junk pre-warm burst shrinks to ~5us of cover;
  - proj tiles 8..15 accumulate chunk ic=1 into SBUF fp32 stages during
    block 0.1, and ic=0 + add during/after 1.1;
  - output is fp16 [T, C] (half the drain traffic); host sums in fp32.
"""

import contextlib
import functools
import sys

sys.path.insert(0, "/opt/trn_rl_repo")

import numpy as np

import concourse.bacc as bacc
import concourse.mybir as mybir
import concourse.tile as tile
from concourse import bass_utils
from concourse import library_config
from concourse.alu_op_type import AluOpType

B, T, C, H, D = 2, 2048, 1024, 16, 64
NEG = -1e10
NCORES = 8
HEADS_PER_CORE = 4
DLOC = HEADS_PER_CORE * D  # 256 local head dims per core
F32 = mybir.dt.float32
F16 = mybir.dt.float16
BF16 = mybir.dt.bfloat16
AF = mybir.ActivationFunctionType

IN_DT = BF16
OUT_DT = F16
# ~duration of junk pre-warm matmuls covering the input-DMA lead-in (ns)
WARM_NS = 6000
# use gpsimd partition_broadcast (attn ucode lib) for the 1/denom broadcast;
# False falls back to a dram round-trip like v1
USE_BCAST = True

NTB = T // 512  # 4 t-blocks in qkv phase
NKC = T // 128  # 16 k-chunks
NQB = 2  # attention q-blocks of 1024
VAUG_W = D + 1  # v columns + ones column (psum row 64 = softmax denominator)


def _pieces(a, end=1024):
    """Split [a, end) at 512-boundaries (psum bank boundaries)."""
    cuts = [a]
    b = (a // 512 + 1) * 512
    while b < end:
        cuts.append(b)
        b += 512
    cuts.append(end)
    return list(zip(cuts[:-1], cuts[1:]))


class Ctx:
    pass


def _emit_prewarm(nc, g):
    """Dependency-free fp32 matmuls covering the input-DMA lead-in so the
    PE hands off at full HAM clock to the dense qkv stream."""
    n = 3 + max(0, int((WARM_NS - 5200) / 880))
    ps = g.pool_x.tile([128, 1024], F32, tag="px", name="warm_ps")
    for i in range(n):
        nc.tensor.matmul(
            ps[:, 0:512],
            g.warm_sb[:, 0:128],
            g.warm_sb,
            start=(i == 0),
            stop=(i == n - 1),
        )
    wsink = g.rnpool.tile([1, 128], F32, tag="wsink", name="wsink", bufs=1)
    nc.vector.tensor_copy(wsink, ps[0:1, 0:128])
    nc.sync.dma_start(out=g.sink_dram.ap()[31:32, 0:128], in_=wsink)


def _emit_qkv_tblock(nc, g, tb):
    """qkv projections for t in [tb*512, (tb+1)*512), chunk-outer."""
    psq = g.pool_x.tile([128, 1024], F32, tag="px", name="psq")
    psk = g.pool_x.tile([128, 1024], F32, tag="px", name="psk")
    psv = [
        g.pool_o.tile([128, 1024], F32, tag="po", name="psvA"),
        g.pool_o.tile([128, 1024], F32, tag="po", name="psvB"),
    ]

    def vslice(ts, width=256):
        return psv[ts // 2][:, (ts % 2) * 512 : (ts % 2) * 512 + width]

    for cc in range(8):
        xts = g.x_sb[cc][:, tb * 512 : (tb + 1) * 512]
        st, sp = cc == 0, cc == 7
        for dt_ in range(2):
            nc.tensor.matmul(
                psq[:, dt_ * 512 : (dt_ + 1) * 512],
                g.wq_sb[cc][:, dt_ * 128 : (dt_ + 1) * 128],
                xts,
                start=st,
                stop=sp,
            )
            nc.tensor.matmul(
                psk[:, dt_ * 512 : (dt_ + 1) * 512],
                g.wk_sb[cc][:, dt_ * 128 : (dt_ + 1) * 128],
                xts,
                start=st,
                stop=sp,
            )
        for ts in range(4):
            nc.tensor.matmul(
                vslice(ts),
                xts[:, ts * 128 : (ts + 1) * 128],
                g.wv_sb[cc],
                start=st,
                stop=sp,
            )
    for dt_ in range(2):
        nc.vector.tensor_scalar(
            out=g.qT_sb[:, dt_, tb * 512 : (tb + 1) * 512],
            in0=psq[:, dt_ * 512 : (dt_ + 1) * 512],
            scalar1=g.bq_sb[:, dt_ : dt_ + 1],
            scalar2=None,
            op0=AluOpType.add,
        )
        nc.vector.tensor_scalar(
            out=g.kT_sb[:, dt_, tb * 512 : (tb + 1) * 512],
            in0=psk[:, dt_ * 512 : (dt_ + 1) * 512],
            scalar1=g.bk_sb[:, dt_ : dt_ + 1],
            scalar2=None,
            op0=AluOpType.add,
        )
    for ts in range(4):
        kc = tb * 4 + ts
        for h in range(4):
            nc.vector.tensor_tensor(
                out=g.vaug[h][:, kc, 0:D],
                in0=vslice(ts)[:, h * D : (h + 1) * D],
                in1=g.bvb_sb[:, h * D : (h + 1) * D],
                op=AluOpType.add,
            )


def _emit_attention_block(nc, g, h, m, braid=None):
    """One head x one 1024-wide q-block of causal attention.

    braid: dict j -> [callable] of filler emitted after O(j-1) at step j.
    Normalize is split per psum bank: bank0 (q cols 0:512 of the block) is
    final after O(8m+3) and normalizes immediately; bank1 at block end.
    """
    prow = (h % 2) * 64
    pi = h // 2
    njs = 8 * m + 8
    last_bank0 = 8 * m + 3
    pso = g.pool_o.tile([128, 1024], F32, tag="po", name="pso")
    uts = {}
    braid = braid or {}

    def emit_S_exp(j):
        a = max(0, 128 * j - 1024 * m)
        pss = g.pool_x.tile([128, 1024], F32, tag="px", name="pss")
        for c0, c1 in _pieces(a):
            nc.tensor.matmul(
                pss[:, c0:c1],
                g.kT_sb[prow : prow + 64, pi, j * 128 : (j + 1) * 128],
                g.qT_sb[prow : prow + 64, pi, m * 1024 + c0 : m * 1024 + c1],
                start=True,
                stop=True,
            )
        ut = g.utpool.tile([128, 1024], BF16, tag="ut", name="ut")
        uts[j] = ut
        nc.scalar.activation(
            out=ut[:, a:1024],
            in_=pss[:, a:1024],
            func=AF.Exp,
            bias=g.mneg_sb[:, j : j + 1],
            scale=0.125,
        )
        if j >= 8 * m:
            nc.vector.tensor_mul(ut[:, a : a + 128], ut[:, a : a + 128], g.tri_sb)

    def emit_O(j):
        a = max(0, 128 * j - 1024 * m)
        ut = uts.pop(j)
        for c0, c1 in _pieces(a):
            stop = j == (last_bank0 if c0 < 512 else njs - 1)
            nc.tensor.matmul(
                pso[0:VAUG_W, c0:c1],
                g.vaug[h][:, j, :],
                ut[:, c0:c1],
                start=(j == 0),
                stop=stop,
            )

    def emit_norm_half(half):
        c0 = half * 512
        rcp = g.rnpool.tile([1, 512], F32, tag="rcp", name="rcp")
        nc.vector.reciprocal(rcp, pso[D : D + 1, c0 : c0 + 512])
        rnb = g.rnpool.tile([64, 512], F32, tag="rnb", name="rnb")
        if USE_BCAST:
            nc.gpsimd.partition_broadcast(rnb, rcp)
        else:
            hmh = (h * NQB + m) * 2 + half
            nc.sync.dma_start(out=g.sink_dram.ap()[hmh : hmh + 1, 0:512], in_=rcp)
            nc.sync.dma_start(
                out=rnb,
                in_=g.sink_dram.ap()[hmh : hmh + 1, 0:512].partition_broadcast(64),
            )
        nc.vector.tensor_tensor(
            out=g.yT_sb[prow : prow + 64, pi, m * 1024 + c0 : m * 1024 + c0 + 512],
            in0=pso[0:D, c0 : c0 + 512],
            in1=rnb,
            op=AluOpType.mult,
        )

    emit_S_exp(0)
    for j in range(1, njs):
        emit_S_exp(j)
        emit_O(j - 1)
        if j - 1 == last_bank0:
            emit_norm_half(0)
        for fn in braid.get(j, ()):
            fn()
    emit_O(njs - 1)
    emit_norm_half(1)
    for fn in braid.get(njs, ()):
        fn()


def _spread(items, j0, j1):
    """Distribute items over j positions [j0, j1] -> dict j -> [item]."""
    out = {}
    n = len(items)
    span = j1 - j0 + 1
    for idx, it in enumerate(items):
        j = j0 + (idx * span) // n
        out.setdefault(j, []).append(it)
    return out


def _build(ctx, nc, tc, ins, out, sink_dram):
    g = Ctx()
    g.sink_dram = sink_dram

    singles = ctx.enter_context(tc.tile_pool(name="singles", bufs=1))
    g.pool_x = ctx.enter_context(tc.tile_pool(name="pool_x", bufs=2, space="PSUM"))
    g.pool_o = ctx.enter_context(tc.tile_pool(name="pool_o", bufs=2, space="PSUM"))
    g.utpool = ctx.enter_context(tc.tile_pool(name="utpool", bufs=6))
    g.rnpool = ctx.enter_context(tc.tile_pool(name="rnpool", bufs=2))
    g.outp = ctx.enter_context(tc.tile_pool(name="outp", bufs=3))

    if USE_BCAST:
        nc.gpsimd.load_library(library_config.attn)

    # tri mask first (sync queue): the pre-warm burst depends only on memset
    g.tri_sb = singles.tile([128, 128], BF16, name="tri_sb")
    nc.sync.dma_start(out=g.tri_sb, in_=ins["tri"].ap())
    g.warm_sb = singles.tile([128, 512], F32, name="warm_sb")
    nc.vector.memset(g.warm_sb, 0.5)
    _emit_prewarm(nc, g)

    # --- inputs: spread over the three DMA-capable queues -----------------
    # (HWDGE on sync/scalar, SWDGE on gpsimd; x chunks avoid gpsimd since
    # the attn-library load runs ahead of them on the Pool queue)
    q_sx = [nc.sync, nc.scalar]
    q3 = [nc.sync, nc.scalar, nc.gpsimd]

    g.x_sb = [singles.tile([128, T], IN_DT, name=f"x{c}") for c in range(8)]
    g.wq_sb = [singles.tile([128, DLOC], IN_DT, name=f"wq{c}") for c in range(8)]
    g.wk_sb = [singles.tile([128, DLOC], IN_DT, name=f"wk{c}") for c in range(8)]
    g.wv_sb = [singles.tile([128, DLOC], IN_DT, name=f"wv{c}") for c in range(8)]
    xT_r = ins["xT"].ap().rearrange("(c p) t -> p c t", p=128)
    wq_src = ins["wq"].ap().rearrange("(c p) m -> p c m", p=128)
    wk_src = ins["wk"].ap().rearrange("(c p) m -> p c m", p=128)
    wv_src = ins["wv"].ap().rearrange("(c p) m -> p c m", p=128)
    for cc in range(8):
        q_sx[cc % 2].dma_start(out=g.x_sb[cc], in_=xT_r[:, cc, :])
        q3[(3 * cc + 0) % 3].dma_start(out=g.wq_sb[cc], in_=wq_src[:, cc, :])
        q3[(3 * cc + 1) % 3].dma_start(out=g.wk_sb[cc], in_=wk_src[:, cc, :])
        q3[(3 * cc + 2) % 3].dma_start(out=g.wv_sb[cc], in_=wv_src[:, cc, :])

    g.bq_sb = singles.tile([128, 2], F32, name="bq_sb")
    g.bk_sb = singles.tile([128, 2], F32, name="bk_sb")
    g.bvb_sb = singles.tile([128, DLOC], F32, name="bvb_sb")
    g.mneg_sb = singles.tile([128, NKC], F32, name="mneg_sb")
    nc.sync.dma_start(out=g.bq_sb, in_=ins["bq"].ap().rearrange("i p -> p i"))
    nc.scalar.dma_start(out=g.bk_sb, in_=ins["bk"].ap().rearrange("i p -> p i"))
    nc.sync.dma_start(out=g.bvb_sb, in_=ins["bv"].ap().partition_broadcast(128))
    nc.scalar.dma_start(out=g.mneg_sb, in_=ins["mneg"].ap())

    # --- persistent activations -----------------------------------------
    g.qT_sb = singles.tile([128, 2, T], BF16, tag="qT", name="qT_sb")
    g.kT_sb = singles.tile([128, 2, T], BF16, tag="kT", name="kT_sb")
    g.vaug = [
        singles.tile([128, NKC, VAUG_W], BF16, tag=f"vaug{h}", name=f"vaug{h}")
        for h in range(4)
    ]
    for h in range(4):
        # col 64 = 1.0: the O^T matmul emits the softmax denominator in
        # psum row 64 for free (32-aligned for engine partition reads)
        nc.vector.memset(g.vaug[h][:, :, D], 1.0)
    g.yT_sb = singles.tile([128, 2, T], IN_DT, tag="yT", name="yT_sb")
    g.stg = {
        i: singles.tile([128, C], F32, name=f"stg{i}") for i in range(8, 16)
    }
    g.obs = {}

    # --- proj braid units -------------------------------------------------
    def ob_for(i):
        if i not in g.obs:
            g.obs[i] = g.outp.tile([128, C], OUT_DT, tag="ob", name=f"ob{i}")
        return g.obs[i]

    def ob_flush(i, queue):
        ob = g.obs.pop(i)
        queue.dma_start(out=out.ap()[i * 128 : (i + 1) * 128, :], in_=ob)

    def proj_full_half(i, hf):
        """tiles 0..7: both chunks of a 512-col output half -> ob fp16."""
        c0 = hf * 512
        psp = g.pool_x.tile([128, 512], F32, tag="px", name="psp")
        for step, ic in enumerate((1, 0)):
            nc.tensor.matmul(
                psp,
                g.yT_sb[:, ic, i * 128 : (i + 1) * 128],
                g.wp_sb[:, ic, c0 : c0 + 512],
                start=(step == 0),
                stop=(step == 1),
            )
        nc.vector.tensor_copy(ob_for(i)[:, c0 : c0 + 512], psp)
        if hf == 1:
            ob_flush(i, nc.gpsimd)

    def stage_half(i, hf):
        """tiles 8..15: chunk ic=1 partial -> fp32 stage."""
        c0 = hf * 512
        psp = g.pool_x.tile([128, 512], F32, tag="px", name="psp")
        nc.tensor.matmul(
            psp,
            g.yT_sb[:, 1, i * 128 : (i + 1) * 128],
            g.wp_sb[:, 1, c0 : c0 + 512],
            start=True,
            stop=True,
        )
        nc.vector.tensor_copy(g.stg[i][:, c0 : c0 + 512], psp)

    def finish_half(i, hf, queue):
        """tiles 8..15: chunk ic=0 + staged ic=1 -> ob fp16."""
        c0 = hf * 512
        psp = g.pool_x.tile([128, 512], F32, tag="px", name="psp")
        nc.tensor.matmul(
            psp,
            g.yT_sb[:, 0, i * 128 : (i + 1) * 128],
            g.wp_sb[:, 0, c0 : c0 + 512],
            start=True,
            stop=True,
        )
        nc.vector.tensor_tensor(
            out=ob_for(i)[:, c0 : c0 + 512],
            in0=psp,
            in1=g.stg[i][:, c0 : c0 + 512],
            op=AluOpType.add,
        )
        if hf == 1:
            ob_flush(i, queue)

    # --- schedule ---------------------------------------------------------
    _emit_qkv_tblock(nc, g, 0)
    _emit_qkv_tblock(nc, g, 1)
    _emit_attention_block(nc, g, 2, 0)
    _emit_qkv_tblock(nc, g, 2)
    _emit_attention_block(nc, g, 3, 0)
    _emit_qkv_tblock(nc, g, 3)
    # c_proj weights (sync queue is idle from here; needed by 2.1's braids)
    g.wp_sb = singles.tile([128, 2, C], IN_DT, name="wp_sb")
    nc.sync.dma_start(
        out=g.wp_sb, in_=ins["wproj"].ap().rearrange("(i p) n -> p i n", p=128)
    )
    _emit_attention_block(nc, g, 0, 0)
    _emit_attention_block(nc, g, 1, 0)

    mk = lambda f, *a: (lambda: f(*a))
    units_03 = [mk(proj_full_half, i, hf) for i in range(0, 4) for hf in (0, 1)]
    units_47 = [mk(proj_full_half, i, hf) for i in range(4, 8) for hf in (0, 1)]
    units_stage = [mk(stage_half, i, hf) for i in range(8, 16) for hf in (0, 1)]
    units_fin_a = [
        mk(finish_half, i, hf, nc.gpsimd) for i in range(8, 12) for hf in (0, 1)
    ]
    _emit_attention_block(nc, g, 2, 1, braid=_spread(units_03, 2, 15))
    _emit_attention_block(nc, g, 3, 1, braid=_spread(units_47, 2, 15))
    _emit_attention_block(nc, g, 0, 1, braid=_spread(units_stage, 2, 15))
    _emit_attention_block(nc, g, 1, 1, braid=_spread(units_fin_a, 13, 16))
    # tail: bank1-half of 1.1 just normalized; finish the last four tiles
    for i in range(12, 16):
        for hf in (0, 1):
            finish_half(i, hf, nc.sync)


@functools.lru_cache(maxsize=1)
def _program():
    nc = bacc.Bacc("TRN2", target_bir_lowering=False, debug=False)
    shapes = {
        "xT": ([C, T], IN_DT),
        "wq": ([C, DLOC], IN_DT),
        "wk": ([C, DLOC], IN_DT),
        "wv": ([C, DLOC], IN_DT),
        "bq": ([2, 128], F32),
        "bk": ([2, 128], F32),
        "bv": ([1, DLOC], F32),
        "wproj": ([DLOC, C], IN_DT),
        "mneg": ([128, NKC], F32),
        "tri": ([128, 128], BF16),
    }
    ins = {
        name: nc.dram_tensor(name, shape, dt_, kind="ExternalInput")
        for name, (shape, dt_) in shapes.items()
    }
    out = nc.dram_tensor("out", [T, C], OUT_DT, kind="ExternalOutput")
    sink_dram = nc.dram_tensor("sink_scratch", [32, 512], F32, kind="Internal")
    with tile.TileContext(nc) as tc, contextlib.ExitStack() as ctx:
        _build(ctx, nc, tc, ins, out, sink_dram)
    nc.compile()
    return nc


def make_in_maps(x, attention_mask, W_attn, b_attn, W_proj, b_proj):
    import ml_dtypes

    in_np = ml_dtypes.bfloat16
    x = np.ascontiguousarray(np.asarray(x, dtype=np.float32))
    attention_mask = np.asarray(attention_mask, dtype=np.float32)
    W_attn = np.asarray(W_attn, dtype=np.float32)
    b_attn = np.asarray(b_attn, dtype=np.float32)
    W_proj = np.asarray(W_proj, dtype=np.float32)

    tri = (np.arange(128)[None, :] >= np.arange(128)[:, None]).astype(np.float32)
    in_maps = []
    for c in range(NCORES):
        b = c // 4
        g = c % 4
        cols = slice(g * DLOC, (g + 1) * DLOC)
        xT = np.ascontiguousarray(x[b].T.astype(in_np))
        mneg = np.ascontiguousarray((attention_mask[b] * NEG).reshape(NKC, 128).T)
        in_maps.append(
            {
                "xT": xT,
                "wq": np.ascontiguousarray(W_attn[:, cols].astype(in_np)),
                "wk": np.ascontiguousarray(W_attn[:, C : 2 * C][:, cols].astype(in_np)),
                "wv": np.ascontiguousarray(
                    W_attn[:, 2 * C : 3 * C][:, cols].astype(in_np)
                ),
                "bq": np.ascontiguousarray(b_attn[cols].reshape(2, 128)),
                "bk": np.ascontiguousarray(b_attn[C : 2 * C][cols].reshape(2, 128)),
                "bv": np.ascontiguousarray(b_attn[2 * C : 3 * C][cols].reshape(1, DLOC)),
                "wproj": np.ascontiguousarray(
                    W_proj[g * DLOC : (g + 1) * DLOC, :].astype(in_np)
                ),
                "mneg": mneg,
                "tri": tri.astype(in_np),
            }
        )
    return in_maps


def kernel(x, attention_mask, W_attn, b_attn, W_proj, b_proj, _res_hook=None):
    in_maps = make_in_maps(x, attention_mask, W_attn, b_attn, W_proj, b_proj)
    nc = _program()
    res = bass_utils.run_bass_kernel_spmd(nc, in_maps, core_ids=list(range(NCORES)))
    if _res_hook is not None:
        _res_hook(res)
    b_proj = np.asarray(b_proj, dtype=np.float32)
    y = np.zeros((B, T, C), dtype=np.float32)
    for c in range(NCORES):
        y[c // 4] += np.asarray(res.results[c]["out"], dtype=np.float32)
    y += b_proj[None, None, :]
    return y
